# revision 7
# baseline (speedup 1.0000x reference)
"""Differential cross-attention head on 8 Trainium2 NeuronCores.

Sharding: data-parallel over batch (4) x sequence-parallel over Tq (2) = 8 cores.
Each core computes out[b, h*1024:(h+1)*1024, :] for (b, h) = divmod(core, 2).

Per-core layout trick: everything is computed in "transposed" orientation so no
on-chip transposes are needed anywhere:
  - host supplies xT = x[b,rows].T            [E, 1024]
  -               encT = encoder_out[b].T     [E, 2048]
  - qT = Wq^T @ xT   (lhsT = Wq chunks, rhs = xT chunks)      [D, 1024]
  - kT = Wk^T @ encT                                          [D, 2048]
  - v  = encT^T @ Wv (lhsT = encT blocks, rhs = Wv chunks)    [Tk, D] natural
  - sT = k1 @ q1T    (lhsT = kT chunk [64,128], rhs = q1T)    [Tk, Tq] transposed
  - eT = exp(sT/8)   (ScalarE, PSUM->SBUF)
  - outT += v_chunk^T @ eT_chunk  (accumulate over Tk chunks) [D, Tq]
  - row-sums r = sum_k eT[k, t]: VectorE accumulates e-chunks, then a
    ones-vector matmul reduces the 128 partitions.
  - out = A1/r1 - lam * A2/r2, written as outT [D, 1024]; host transposes back.
"""

import sys
from contextlib import ExitStack

import numpy as np

_TRN_REPO = "/opt/trn_rl_repo"
if _TRN_REPO not in sys.path:
    sys.path.insert(0, _TRN_REPO)

import concourse.bass as bass
import concourse.tile as tile
from concourse import mybir
from concourse.bass import ts

F32 = mybir.dt.float32
BF16 = mybir.dt.bfloat16

E = 1024          # embed dim
D = 128           # head dim
HD = 64           # half head dim
B = 4
TQ = 2048
TK = 2048
NCORES = 8
TQL = B * TQ // NCORES   # 1024 query rows per core
EC = E // 128            # 8 contraction chunks for projections
NG = TQL // 512          # 2 query groups of 512
KC = TK // 128           # 16 key chunks
SCALE = 0.125            # 1/sqrt(64)

# dtype knobs (flip to BF16 for perf experiments)
DT_IN = np.float32       # host-side dtype of xT / encT
DT_QK = F32              # qT / kT sbuf dtype (QK^T matmul operands)
DT_E = F32               # exp(s) tiles and v sbuf dtype (PV matmul operands)
ROW_PACK = True          # run s1/s2 QK matmuls concurrently in PE row groups

_DT_IN_MYBIR = {np.float32: F32}


def _np_to_mybir(dt):
    import ml_dtypes

    if dt == np.float32:
        return F32
    if dt == ml_dtypes.bfloat16:
        return BF16
    raise ValueError(dt)


def _build(nc: bass.Bass):
    dt_in = _np_to_mybir(DT_IN)
    xT = nc.dram_tensor("xT", [E, TQL], dt_in, kind="ExternalInput").ap()
    encT = nc.dram_tensor("encT", [E, TK], dt_in, kind="ExternalInput").ap()
    wq = nc.dram_tensor("wq", [E, D], F32, kind="ExternalInput").ap()
    wk = nc.dram_tensor("wk", [E, D], F32, kind="ExternalInput").ap()
    wv = nc.dram_tensor("wv", [E, D], F32, kind="ExternalInput").ap()
    bq = nc.dram_tensor("bq", [D], F32, kind="ExternalInput").ap()
    bk = nc.dram_tensor("bk", [D], F32, kind="ExternalInput").ap()
    bv = nc.dram_tensor("bv", [D], F32, kind="ExternalInput").ap()
    lam = nc.dram_tensor("lam", [1, 1], F32, kind="ExternalInput").ap()
    outT = nc.dram_tensor("outT", [D, TQL], F32, kind="ExternalOutput").ap()

    xT_r = xT.rearrange("(c p) t -> c p t", p=128)      # [EC, 128, TQL]
    encT_r = encT.rearrange("(c p) t -> c p t", p=128)  # [EC, 128, TK]

    Exp = mybir.ActivationFunctionType.Exp

    with tile.TileContext(nc) as tc, ExitStack() as ctx:
        const = ctx.enter_context(tc.tile_pool(name="const", bufs=1))
        enc_pool = ctx.enter_context(tc.tile_pool(name="enc", bufs=1))
        stream = ctx.enter_context(tc.tile_pool(name="stream", bufs=4))
        proj = ctx.enter_context(tc.tile_pool(name="proj", bufs=1))
        epool = ctx.enter_context(tc.tile_pool(name="epool", bufs=6))
        rpool = ctx.enter_context(tc.tile_pool(name="rpool", bufs=4))
        small = ctx.enter_context(tc.tile_pool(name="small", bufs=4))
        outp = ctx.enter_context(tc.tile_pool(name="outp", bufs=4))
        psS = ctx.enter_context(tc.tile_pool(name="psS", bufs=4, space="PSUM"))
        psPV = ctx.enter_context(tc.tile_pool(name="psPV", bufs=2, space="PSUM"))
        psR = ctx.enter_context(tc.tile_pool(name="psR", bufs=2, space="PSUM"))

        # ---- constants ----
        wq_sb = const.tile([128, EC, D], F32, tag="wq")
        nc.sync.dma_start(out=wq_sb, in_=wq.rearrange("(c p) d -> p c d", p=128))
        wk_sb = const.tile([128, EC, D], F32, tag="wk")
        nc.sync.dma_start(out=wk_sb, in_=wk.rearrange("(c p) d -> p c d", p=128))
        wv_sb = const.tile([128, EC, D], F32, tag="wv")
        nc.sync.dma_start(out=wv_sb, in_=wv.rearrange("(c p) d -> p c d", p=128))
        bq_sb = const.tile([128, 1], F32, tag="bq")
        nc.sync.dma_start(out=bq_sb, in_=bq.rearrange("(p o) -> p o", o=1))
        bk_sb = const.tile([128, 1], F32, tag="bk")
        nc.sync.dma_start(out=bk_sb, in_=bk.rearrange("(p o) -> p o", o=1))
        bv_sb = const.tile([1, D], F32, tag="bv")
        nc.sync.dma_start(out=bv_sb, in_=bv.rearrange("(o d) -> o d", o=1))
        lam_sb = const.tile([1, 1], F32, tag="lam")
        nc.sync.dma_start(out=lam_sb, in_=lam)
        ones_col = const.tile([128, 1], F32, tag="ones_col")
        nc.vector.memset(ones_col, 1.0)
        ones_row = const.tile([1, 128], F32, tag="ones_row")
        nc.vector.memset(ones_row, 1.0)

        # ---- stage encoder_out^T in SBUF (used by both k and v projections) ----
        enc_sb = enc_pool.tile([128, EC, TK], dt_in, tag="enc")
        for c in range(EC):
            nc.sync.dma_start(out=enc_sb[:, c, :], in_=encT_r[c])

        # ---- q^T projection: qT[D, TQL] = Wq^T @ x^T  (+ bq per partition) ----
        qT_sb = proj.tile([128, TQL], DT_QK, tag="qT")
        for g in range(NG):
            qp = psS.tile([128, 512], F32, tag="ps_s")
            for c in range(EC):
                xq = stream.tile([128, 512], dt_in, tag="xq")
                nc.sync.dma_start(out=xq, in_=xT_r[c, :, ts(g, 512)])
                nc.tensor.matmul(qp, lhsT=wq_sb[:, c, :], rhs=xq,
                                 start=(c == 0), stop=(c == EC - 1))
            nc.scalar.add(qT_sb[:, ts(g, 512)], qp, add=bq_sb)

        # ---- k^T projection: kT[D, TK] = Wk^T @ enc^T (+ bk) ----
        kT_sb = proj.tile([128, TK], DT_QK, tag="kT")
        for g in range(TK // 512):
            kp = psS.tile([128, 512], F32, tag="ps_s")
            for c in range(EC):
                nc.tensor.matmul(kp, lhsT=wk_sb[:, c, :],
                                 rhs=enc_sb[:, c, ts(g, 512)],
                                 start=(c == 0), stop=(c == EC - 1))
            nc.scalar.add(kT_sb[:, ts(g, 512)], kp, add=bk_sb)

        # ---- v projection (natural layout): v[Tk, D] = enc @ Wv + bv ----
        # bias enters via a K=1 ones-matmul that initializes PSUM.
        v_sb = proj.tile([128, KC, D], DT_E, tag="v")
        for t in range(KC):
            vp = psS.tile([128, D], F32, tag="ps_s")
            nc.tensor.matmul(vp, lhsT=ones_row, rhs=bv_sb, start=True, stop=False)
            for c in range(EC):
                nc.tensor.matmul(vp, lhsT=enc_sb[:, c, ts(t, 128)],
                                 rhs=wv_sb[:, c, :],
                                 start=False, stop=(c == EC - 1))
            nc.scalar.copy(v_sb[:, t, :], vp)

        # ---- attention, one group of 512 queries at a time ----
        for g in range(NG):
            pv1 = psPV.tile([128, 512], F32, tag="ps_pv")
            pv2 = psPV.tile([128, 512], F32, tag="ps_pv")
            racc1 = rpool.tile([128, 512], F32, tag="racc")
            racc2 = rpool.tile([128, 512], F32, tag="racc")
            for k in range(KC):
                s1 = psS.tile([128, 512], F32, tag="ps_s")
                s2 = psS.tile([128, 512], F32, tag="ps_s")
                if ROW_PACK:
                    nc.tensor.matmul(s1, lhsT=kT_sb[0:64, ts(k, 128)],
                                     rhs=qT_sb[0:64, ts(g, 512)],
                                     start=True, stop=True, tile_position=(0, 0))
                    nc.tensor.matmul(s2, lhsT=kT_sb[64:128, ts(k, 128)],
                                     rhs=qT_sb[64:128, ts(g, 512)],
                                     start=True, stop=True, tile_position=(64, 0))
                else:
                    nc.tensor.matmul(s1, lhsT=kT_sb[0:64, ts(k, 128)],
                                     rhs=qT_sb[0:64, ts(g, 512)],
                                     start=True, stop=True)
                    nc.tensor.matmul(s2, lhsT=kT_sb[64:128, ts(k, 128)],
                                     rhs=qT_sb[64:128, ts(g, 512)],
                                     start=True, stop=True)
                e1 = epool.tile([128, 512], DT_E, tag="e")
                nc.scalar.activation(e1, s1, Exp, scale=SCALE)
                e2 = epool.tile([128, 512], DT_E, tag="e")
                nc.scalar.activation(e2, s2, Exp, scale=SCALE)
                # out^T accumulation: pv += v_chunk^T @ e_chunk
                nc.tensor.matmul(pv1, lhsT=v_sb[:, k, :], rhs=e1,
                                 start=(k == 0), stop=(k == KC - 1),
                                 skip_group_check=True)
                nc.tensor.matmul(pv2, lhsT=v_sb[:, k, :], rhs=e2,
                                 start=(k == 0), stop=(k == KC - 1),
                                 skip_group_check=True)
                # row-sum accumulators (partition dim reduced later on PE)
                if k == 0:
                    nc.vector.tensor_copy(racc1, e1)
                    nc.vector.tensor_copy(racc2, e2)
                else:
                    nc.vector.tensor_add(racc1, racc1, e1)
                    nc.vector.tensor_add(racc2, racc2, e2)

            # softmax denominators: reduce 128 partitions with a ones-matmul
            r1p = psR.tile([1, 512], F32, tag="ps_r")
            nc.tensor.matmul(r1p, lhsT=ones_col, rhs=racc1, start=True, stop=True)
            r2p = psR.tile([1, 512], F32, tag="ps_r")
            nc.tensor.matmul(r2p, lhsT=ones_col, rhs=racc2, start=True, stop=True)
            rrec1 = small.tile([1, 512], F32, tag="rrec")
            nc.vector.reciprocal(rrec1, r1p)
            rrec2 = small.tile([1, 512], F32, tag="rrec")
            nc.vector.reciprocal(rrec2, r2p)
            # fold lam into 1/r2
            nc.vector.tensor_scalar_mul(rrec2, rrec2, lam_sb)
            # broadcast 1/r across partitions: K=1 ones-matmul into PSUM,
            # then ScalarE copy to SBUF (SBUF APs can't have step-0 partitions)
            rb1p = psR.tile([128, 512], F32, tag="ps_r")
            nc.tensor.matmul(rb1p, lhsT=ones_row, rhs=rrec1, start=True, stop=True)
            rb1 = outp.tile([128, 512], F32, tag="rb")
            nc.scalar.copy(rb1, rb1p)
            rb2p = psR.tile([128, 512], F32, tag="ps_r")
            nc.tensor.matmul(rb2p, lhsT=ones_row, rhs=rrec2, start=True, stop=True)
            rb2 = outp.tile([128, 512], F32, tag="rb")
            nc.scalar.copy(rb2, rb2p)
            t1 = outp.tile([128, 512], F32, tag="t")
            nc.vector.tensor_mul(t1, pv1, rb1)
            t2 = outp.tile([128, 512], F32, tag="t")
            nc.vector.tensor_mul(t2, pv2, rb2)
            o = outp.tile([128, 512], F32, tag="o")
            nc.vector.tensor_sub(o, t1, t2)
            nc.sync.dma_start(out=outT[:, ts(g, 512)], in_=o)

    return nc


def _make_bass():
    from concourse import bacc

    nc = bacc.Bacc("TRN2", target_bir_lowering=False, debug=False)
    _build(nc)
    nc.compile()
    return nc


_nc_cache = None


def kernel(x, encoder_out, W_q, b_q, W_k, b_k, W_v, b_v,
           lambda_q1, lambda_k1, lambda_q2, lambda_k2, lambda_init):
    global _nc_cache
    from concourse import bass_utils

    x = np.asarray(x, np.float32)
    encoder_out = np.asarray(encoder_out, np.float32)
    W_q = np.asarray(W_q, np.float32)
    W_k = np.asarray(W_k, np.float32)
    W_v = np.asarray(W_v, np.float32)
    b_q = np.asarray(b_q, np.float32)
    b_k = np.asarray(b_k, np.float32)
    b_v = np.asarray(b_v, np.float32)

    lam = np.float32(
        np.exp(np.float32(np.asarray(lambda_q1, np.float32)
                          @ np.asarray(lambda_k1, np.float32)))
        - np.exp(np.float32(np.asarray(lambda_q2, np.float32)
                            @ np.asarray(lambda_k2, np.float32)))
        + np.float32(np.asarray(lambda_init, np.float32))
    )
    lam_arr = np.full((1, 1), lam, np.float32)

    if _nc_cache is None:
        _nc_cache = _make_bass()
    nc = _nc_cache

    encTs = [np.ascontiguousarray(encoder_out[b].T).astype(DT_IN) for b in range(B)]
    in_maps = []
    for c in range(NCORES):
        b, h = divmod(c, 2)
        xTs = np.ascontiguousarray(x[b, h * TQL:(h + 1) * TQL, :].T).astype(DT_IN)
        in_maps.append({
            "xT": xTs, "encT": encTs[b],
            "wq": W_q, "wk": W_k, "wv": W_v,
            "bq": b_q, "bk": b_k, "bv": b_v,
            "lam": lam_arr,
        })

    res = bass_utils.run_bass_kernel_spmd(nc, in_maps, core_ids=list(range(NCORES)))
    kernel.last_result = res

    out = np.empty((B, TQ, D), np.float32)
    for c in range(NCORES):
        b, h = divmod(c, 2)
        out[b, h * TQL:(h + 1) * TQL, :] = res.results[c]["outT"].T
    return out


# revision 14
# speedup vs baseline: 2.1502x; 2.1502x over previous
"""Differential cross-attention head on 8 Trainium2 NeuronCores.

Sharding: data-parallel over batch (4) x sequence-parallel over Tq (2) = 8 cores.
Each core computes out[b, h*1024:(h+1)*1024, :] for (b, h) = divmod(core, 2).

Per-core math is laid out in "transposed" orientation so no on-chip transposes
are needed anywhere (host supplies xT/encT, host transposes the output back):
  - qT = Wq^T @ xT            [D, 1024]   (lhsT = Wq chunks, rhs = xT chunks)
  - kT = Wk^T @ encT          [D, Tk]     (produced per 512-wide Tk group)
  - v  = encT^T @ Wv          [Tk, D]     natural (lhsT = encT blocks)
  - s^T = k @ q^T             [Tk, Tq]    scores transposed; s1|s2 packed into
                                          one [128,1024] PSUM tile via PE
                                          row-group tiling (K=64 each)
  - e^T = exp(s^T/8)          ScalarE, PSUM->SBUF, bf16
  - out^T += v_chunk^T @ e^T  one N=1024 matmul accumulates A1|A2
  - row-sums r: VectorE accumulates e-chunks, ones-matmul reduces partitions
  - out = A1/r1 - lam*A2/r2

The whole kernel streams over Tk groups of 512 (k-proj -> v-proj -> QK ->
exp -> PV), so DMA, projections and attention overlap and the PE stays busy
(keeps the HAM clock-gate at 8/8).
"""

import sys
from contextlib import ExitStack

import numpy as np

_TRN_REPO = "/opt/trn_rl_repo"
if _TRN_REPO not in sys.path:
    sys.path.insert(0, _TRN_REPO)

import ml_dtypes

import concourse.bass as bass
import concourse.tile as tile
from concourse import mybir
from concourse.bass import ts

F32 = mybir.dt.float32
BF16 = mybir.dt.bfloat16

E = 1024          # embed dim
D = 128           # head dim
B = 4
TQ = 2048
TK = 2048
NCORES = 8
TQL = B * TQ // NCORES   # 1024 query rows per core
EC = E // 128            # 8 contraction chunks for projections
NG = TQL // 512          # 2 query groups of 512
TKG = TK // 512          # 4 Tk groups
SCALE = 0.125            # 1/sqrt(64)

NP_BF16 = ml_dtypes.bfloat16

# dtype knobs
DT_IN = NP_BF16          # host-side dtype of xT / encT / weights
DT_QK = BF16             # qT / kT sbuf dtype (QK^T matmul operands)
DT_E = BF16              # exp(s) tiles and v sbuf dtype (PV matmul operands)


def _np_to_mybir(dt):
    if dt == np.float32:
        return F32
    if dt == NP_BF16:
        return BF16
    raise ValueError(dt)


def _build(nc: bass.Bass):
    dt_in = _np_to_mybir(DT_IN)
    xT = nc.dram_tensor("xT", [E, TQL], dt_in, kind="ExternalInput").ap()
    encT = nc.dram_tensor("encT", [E, TK], dt_in, kind="ExternalInput").ap()
    wq = nc.dram_tensor("wq", [E, D], dt_in, kind="ExternalInput").ap()
    wk = nc.dram_tensor("wk", [E, D], dt_in, kind="ExternalInput").ap()
    wv = nc.dram_tensor("wv", [E, D], dt_in, kind="ExternalInput").ap()
    bq = nc.dram_tensor("bq", [D], F32, kind="ExternalInput").ap()
    bk = nc.dram_tensor("bk", [D], F32, kind="ExternalInput").ap()
    bv = nc.dram_tensor("bv", [D], F32, kind="ExternalInput").ap()
    lam = nc.dram_tensor("lam", [1, 1], F32, kind="ExternalInput").ap()
    outT = nc.dram_tensor("outT", [D, TQL], F32, kind="ExternalOutput").ap()

    xT_r = xT.rearrange("(c p) t -> c p t", p=128)      # [EC, 128, TQL]
    encT_r = encT.rearrange("(c p) t -> c p t", p=128)  # [EC, 128, TK]

    Exp = mybir.ActivationFunctionType.Exp

    with tile.TileContext(nc) as tc, ExitStack() as ctx:
        const = ctx.enter_context(tc.tile_pool(name="const", bufs=1))
        stream = ctx.enter_context(tc.tile_pool(name="stream", bufs=4))
        encp = ctx.enter_context(tc.tile_pool(name="encp", bufs=2))
        proj = ctx.enter_context(tc.tile_pool(name="proj", bufs=1))
        kvp = ctx.enter_context(tc.tile_pool(name="kvp", bufs=2))
        epool = ctx.enter_context(tc.tile_pool(name="epool", bufs=6))
        rpool = ctx.enter_context(tc.tile_pool(name="rpool", bufs=2))
        small = ctx.enter_context(tc.tile_pool(name="small", bufs=4))
        outp = ctx.enter_context(tc.tile_pool(name="outp", bufs=4))
        psS = ctx.enter_context(tc.tile_pool(name="psS", bufs=2, space="PSUM"))
        psPV = ctx.enter_context(tc.tile_pool(name="psPV", bufs=2, space="PSUM"))

        # ---- constants ----
        wq_sb = const.tile([128, EC, D], dt_in, tag="wq")
        nc.sync.dma_start(out=wq_sb, in_=wq.rearrange("(c p) d -> p c d", p=128))
        wk_sb = const.tile([128, EC, D], dt_in, tag="wk")
        nc.sync.dma_start(out=wk_sb, in_=wk.rearrange("(c p) d -> p c d", p=128))
        wv_sb = const.tile([128, EC, D], dt_in, tag="wv")
        nc.sync.dma_start(out=wv_sb, in_=wv.rearrange("(c p) d -> p c d", p=128))
        bq_sb = const.tile([128, 1], F32, tag="bq")
        nc.sync.dma_start(out=bq_sb, in_=bq.rearrange("(p o) -> p o", o=1))
        bk_sb = const.tile([128, 1], F32, tag="bk")
        nc.sync.dma_start(out=bk_sb, in_=bk.rearrange("(p o) -> p o", o=1))
        bv_sb = const.tile([1, D], F32, tag="bv")
        nc.sync.dma_start(out=bv_sb, in_=bv.rearrange("(o d) -> o d", o=1))
        lam_sb = const.tile([1, 1], F32, tag="lam")
        nc.sync.dma_start(out=lam_sb, in_=lam)
        ones_col = const.tile([128, 1], F32, tag="ones_col")
        nc.vector.memset(ones_col, 1.0)
        ones_row_f32 = const.tile([1, 128], F32, tag="ones_row_f32")
        nc.vector.memset(ones_row_f32, 1.0)

        # ---- q^T projection: qT[D, TQL] = Wq^T @ x^T (+ bq) ----
        qT_sb = proj.tile([128, TQL], DT_QK, tag="qT")
        for g in range(NG):
            qp = psS.tile([128, 1024], F32, tag="ps_s")
            for c in range(EC):
                xq = stream.tile([128, 512], dt_in, tag="xq")
                nc.sync.dma_start(out=xq, in_=xT_r[c, :, ts(g, 512)])
                nc.tensor.matmul(qp[:, 0:512], lhsT=wq_sb[:, c, :], rhs=xq,
                                 start=(c == 0), stop=(c == EC - 1))
            nc.scalar.add(qT_sb[:, ts(g, 512)], qp[:, 0:512], add=bq_sb)

        # persistent attention state (accumulated across all Tk groups)
        pv = [psPV.tile([128, 1024], F32, tag="ps_pv", name=f"pv{g}")
              for g in range(NG)]
        racc = [rpool.tile([128, 1024], F32, tag="racc", name=f"racc{g}")
                for g in range(NG)]

        # ---- stream over Tk groups: k-proj, v-proj, QK, exp, PV ----
        for tg in range(TKG):
            # encoder slice for this group: [128, EC, 512]
            enc_sb = encp.tile([128, EC, 512], dt_in, tag="enc")
            for c in range(EC):
                nc.sync.dma_start(out=enc_sb[:, c, :], in_=encT_r[c, :, ts(tg, 512)])

            # k^T for this group: [D, 512]
            kp = psS.tile([128, 1024], F32, tag="ps_s")
            for c in range(EC):
                nc.tensor.matmul(kp[:, 0:512], lhsT=wk_sb[:, c, :],
                                 rhs=enc_sb[:, c, :],
                                 start=(c == 0), stop=(c == EC - 1))
            kT_blk = kvp.tile([128, 512], DT_QK, tag="kT")
            nc.scalar.add(kT_blk, kp[:, 0:512], add=bk_sb)

            # v (natural) for this group: 4 blocks of [128, 128]
            v_blk = kvp.tile([128, 4, D], DT_E, tag="v")
            for t in range(4):
                vp = psS.tile([128, 1024], F32, tag="ps_s")
                nc.tensor.matmul(vp[:, 0:D], lhsT=ones_row_f32, rhs=bv_sb,
                                 start=True, stop=False)
                for c in range(EC):
                    nc.tensor.matmul(vp[:, 0:D],
                                     lhsT=enc_sb[:, c, ts(t, 128)],
                                     rhs=wv_sb[:, c, :],
                                     start=False, stop=(c == EC - 1))
                nc.scalar.copy(v_blk[:, t, :], vp[:, 0:D])

            # attention for the 4 chunks of this Tk group, both q groups
            for kc in range(4):
                k_glob = tg * 4 + kc
                for g in range(NG):
                    s12 = psS.tile([128, 1024], F32, tag="ps_s")
                    nc.tensor.matmul(s12[:, 0:512],
                                     lhsT=kT_blk[0:64, ts(kc, 128)],
                                     rhs=qT_sb[0:64, ts(g, 512)],
                                     start=True, stop=True, tile_position=(0, 0))
                    nc.tensor.matmul(s12[:, 512:1024],
                                     lhsT=kT_blk[64:128, ts(kc, 128)],
                                     rhs=qT_sb[64:128, ts(g, 512)],
                                     start=True, stop=True, tile_position=(64, 0))
                    e12 = epool.tile([128, 1024], DT_E, tag="e")
                    nc.scalar.activation(e12, s12, Exp, scale=SCALE)
                    for h in range(2):
                        nc.tensor.matmul(pv[g][:, ts(h, 512)],
                                         lhsT=v_blk[:, kc, :],
                                         rhs=e12[:, ts(h, 512)],
                                         start=(k_glob == 0),
                                         stop=(k_glob == TK // 128 - 1),
                                         skip_group_check=True)
                    if k_glob == 0:
                        nc.vector.tensor_copy(racc[g], e12)
                    else:
                        nc.vector.tensor_add(racc[g], racc[g], e12)

        # ---- softmax denominators + combine ----
        for g in range(NG):
            r12p = psS.tile([1, 1024], F32, tag="ps_s")
            for h in range(2):
                nc.tensor.matmul(r12p[:, ts(h, 512)], lhsT=ones_col,
                                 rhs=racc[g][:, ts(h, 512)], start=True, stop=True)
            rrec = small.tile([1, 1024], F32, tag="rrec")
            nc.vector.reciprocal(rrec, r12p)
            nc.vector.tensor_scalar_mul(rrec[:, 512:1024], rrec[:, 512:1024], lam_sb)
            rbp = psS.tile([128, 1024], F32, tag="ps_s")
            # broadcast 1/r across partitions via K=1 ones-matmul
            for h in range(2):
                nc.tensor.matmul(rbp[:, ts(h, 512)], lhsT=ones_row_f32,
                                 rhs=rrec[:, ts(h, 512)], start=True, stop=True)
            rb = outp.tile([128, 1024], F32, tag="rb")
            nc.scalar.copy(rb, rbp)
            m12 = outp.tile([128, 1024], F32, tag="m12")
            nc.vector.tensor_mul(m12, pv[g], rb)
            o = outp.tile([128, 512], F32, tag="o")
            nc.vector.tensor_sub(o, m12[:, 0:512], m12[:, 512:1024])
            nc.sync.dma_start(out=outT[:, ts(g, 512)], in_=o)

    return nc


def _make_bass():
    from concourse import bacc

    nc = bacc.Bacc("TRN2", target_bir_lowering=False, debug=False)
    _build(nc)
    nc.compile()
    return nc


_nc_cache = None


def kernel(x, encoder_out, W_q, b_q, W_k, b_k, W_v, b_v,
           lambda_q1, lambda_k1, lambda_q2, lambda_k2, lambda_init):
    global _nc_cache
    from concourse import bass_utils

    x = np.asarray(x, np.float32)
    encoder_out = np.asarray(encoder_out, np.float32)
    W_q = np.asarray(W_q, np.float32).astype(DT_IN)
    W_k = np.asarray(W_k, np.float32).astype(DT_IN)
    W_v = np.asarray(W_v, np.float32).astype(DT_IN)
    b_q = np.asarray(b_q, np.float32)
    b_k = np.asarray(b_k, np.float32)
    b_v = np.asarray(b_v, np.float32)

    lam = np.float32(
        np.exp(np.float32(np.asarray(lambda_q1, np.float32)
                          @ np.asarray(lambda_k1, np.float32)))
        - np.exp(np.float32(np.asarray(lambda_q2, np.float32)
                            @ np.asarray(lambda_k2, np.float32)))
        + np.float32(np.asarray(lambda_init, np.float32))
    )
    lam_arr = np.full((1, 1), lam, np.float32)

    if _nc_cache is None:
        _nc_cache = _make_bass()
    nc = _nc_cache

    encTs = [np.ascontiguousarray(encoder_out[b].T).astype(DT_IN) for b in range(B)]
    in_maps = []
    for c in range(NCORES):
        b, h = divmod(c, 2)
        xTs = np.ascontiguousarray(x[b, h * TQL:(h + 1) * TQL, :].T).astype(DT_IN)
        in_maps.append({
            "xT": xTs, "encT": encTs[b],
            "wq": W_q, "wk": W_k, "wv": W_v,
            "bq": b_q, "bk": b_k, "bv": b_v,
            "lam": lam_arr,
        })

    res = bass_utils.run_bass_kernel_spmd(nc, in_maps, core_ids=list(range(NCORES)))
    kernel.last_result = res

    out = np.empty((B, TQ, D), np.float32)
    for c in range(NCORES):
        b, h = divmod(c, 2)
        out[b, h * TQL:(h + 1) * TQL, :] = res.results[c]["outT"].T
    return out


# revision 16
# speedup vs baseline: 2.4670x; 1.1473x over previous
"""Differential cross-attention head on 8 Trainium2 NeuronCores.

Sharding: data-parallel over batch (4) x sequence-parallel over Tq (2) = 8 cores.
Each core computes out[b, h*1024:(h+1)*1024, :] for (b, h) = divmod(core, 2).

Per-core math is laid out in "transposed" orientation so no on-chip transposes
are needed anywhere (host supplies xT/encT, host transposes the output back):
  - qT = Wq^T @ xT            [D, 1024]   (lhsT = Wq chunks, rhs = xT chunks)
  - kT = Wk^T @ encT          [D, Tk]     (produced per 512-wide Tk group)
  - v  = encT^T @ Wv          [Tk, D]     natural (lhsT = encT blocks)
  - s^T = k @ q^T             [Tk, Tq]    scores transposed; s1|s2 packed into
                                          one [128,1024] PSUM tile via PE
                                          row-group tiling (K=64 each, runs
                                          concurrently in the array)
  - e^T = exp(s^T/8)          ScalarE, PSUM->SBUF, bf16
  - A^T += v_chunk^T @ e^T    accumulated in PSUM ([A1|A2] per q group)
  - row-sums r: VectorE accumulates e-chunks, ones-matmul reduces partitions
The normalization out = A1/r1 - lam*A2/r2 (1M cheap elementwise ops) and the
final transpose happen on the host; A and r stream out via DMA.

Group-0 attention is interleaved with the k/v projections of each Tk group so
DMA, projections and attention overlap; group-1 runs as a pure steady phase.
"""

import sys
from contextlib import ExitStack

import numpy as np

_TRN_REPO = "/opt/trn_rl_repo"
if _TRN_REPO not in sys.path:
    sys.path.insert(0, _TRN_REPO)

import ml_dtypes

import concourse.bass as bass
import concourse.tile as tile
from concourse import mybir
from concourse.bass import ts

F32 = mybir.dt.float32
BF16 = mybir.dt.bfloat16

E = 1024          # embed dim
D = 128           # head dim
B = 4
TQ = 2048
TK = 2048
NCORES = 8
TQL = B * TQ // NCORES   # 1024 query rows per core
EC = E // 128            # 8 contraction chunks for projections
NG = TQL // 512          # 2 query groups of 512
TKG = TK // 512          # 4 Tk groups
KC = TK // 128           # 16 Tk chunks
SCALE = 0.125            # 1/sqrt(64)

NP_BF16 = ml_dtypes.bfloat16

# dtype knobs
DT_IN = NP_BF16          # host-side dtype of xT / encT / weights
DT_QK = BF16             # qT / kT sbuf dtype (QK^T matmul operands)
DT_E = BF16              # exp(s) tiles and v sbuf dtype (PV matmul operands)


def _np_to_mybir(dt):
    if dt == np.float32:
        return F32
    if dt == NP_BF16:
        return BF16
    raise ValueError(dt)


def _build(nc: bass.Bass, with_vbias: bool):
    dt_in = _np_to_mybir(DT_IN)
    xT = nc.dram_tensor("xT", [E, TQL], dt_in, kind="ExternalInput").ap()
    encT = nc.dram_tensor("encT", [E, TK], dt_in, kind="ExternalInput").ap()
    wq = nc.dram_tensor("wq", [E, D], dt_in, kind="ExternalInput").ap()
    wk = nc.dram_tensor("wk", [E, D], dt_in, kind="ExternalInput").ap()
    wv = nc.dram_tensor("wv", [E, D], dt_in, kind="ExternalInput").ap()
    bq = nc.dram_tensor("bq", [D], F32, kind="ExternalInput").ap()
    bk = nc.dram_tensor("bk", [D], F32, kind="ExternalInput").ap()
    bv = nc.dram_tensor("bv", [D], F32, kind="ExternalInput").ap()
    pvd = nc.dram_tensor("pvd", [D, NG * 1024], F32, kind="ExternalOutput").ap()
    rd = nc.dram_tensor("rd", [NG, 1024], F32, kind="ExternalOutput").ap()

    xT_r = xT.rearrange("(c p) t -> c p t", p=128)      # [EC, 128, TQL]
    encT_r = encT.rearrange("(c p) t -> c p t", p=128)  # [EC, 128, TK]

    Exp = mybir.ActivationFunctionType.Exp

    with tile.TileContext(nc) as tc, ExitStack() as ctx:
        const = ctx.enter_context(tc.tile_pool(name="const", bufs=1))
        stream = ctx.enter_context(tc.tile_pool(name="stream", bufs=4))
        encpool = ctx.enter_context(tc.tile_pool(name="encpool", bufs=1))
        proj = ctx.enter_context(tc.tile_pool(name="proj", bufs=1))
        epool = ctx.enter_context(tc.tile_pool(name="epool", bufs=6))
        rpool = ctx.enter_context(tc.tile_pool(name="rpool", bufs=2))
        psS = ctx.enter_context(tc.tile_pool(name="psS", bufs=2, space="PSUM"))
        psPV = ctx.enter_context(tc.tile_pool(name="psPV", bufs=2, space="PSUM"))

        # ---- constants ----
        wq_sb = const.tile([128, EC, D], dt_in, tag="wq")
        nc.sync.dma_start(out=wq_sb, in_=wq.rearrange("(c p) d -> p c d", p=128))
        wk_sb = const.tile([128, EC, D], dt_in, tag="wk")
        nc.sync.dma_start(out=wk_sb, in_=wk.rearrange("(c p) d -> p c d", p=128))
        wv_sb = const.tile([128, EC, D], dt_in, tag="wv")
        nc.sync.dma_start(out=wv_sb, in_=wv.rearrange("(c p) d -> p c d", p=128))
        bq_sb = const.tile([128, 1], F32, tag="bq")
        nc.sync.dma_start(out=bq_sb, in_=bq.rearrange("(p o) -> p o", o=1))
        bk_sb = const.tile([128, 1], F32, tag="bk")
        nc.sync.dma_start(out=bk_sb, in_=bk.rearrange("(p o) -> p o", o=1))
        if with_vbias:
            bv_sb = const.tile([1, D], F32, tag="bv")
            nc.sync.dma_start(out=bv_sb, in_=bv.rearrange("(o d) -> o d", o=1))
            ones_row_f32 = const.tile([1, 128], F32, tag="ones_row_f32")
            nc.vector.memset(ones_row_f32, 1.0)
        ones_col = const.tile([128, 1], F32, tag="ones_col")
        nc.vector.memset(ones_col, 1.0)

        # ---- q^T projection: qT[D, TQL] = Wq^T @ x^T (+ bq) ----
        qT_sb = proj.tile([128, TQL], DT_QK, tag="qT")
        for g in range(NG):
            qp = psS.tile([128, 1024], F32, tag="ps_s")
            for c in range(EC):
                xq = stream.tile([128, 512], dt_in, tag="xq")
                nc.sync.dma_start(out=xq, in_=xT_r[c, :, ts(g, 512)])
                nc.tensor.matmul(qp[:, 0:512], lhsT=wq_sb[:, c, :], rhs=xq,
                                 start=(c == 0), stop=(c == EC - 1))
            nc.scalar.add(qT_sb[:, ts(g, 512)], qp[:, 0:512], add=bq_sb)

        # stage full encoder^T (bf16, 32KB/partition) — all DMAs go out early
        enc_sb = encpool.tile([128, EC, TK], dt_in, tag="enc")
        for c in range(EC):
            for tg in range(TKG):
                nc.sync.dma_start(out=enc_sb[:, c, ts(tg, 512)],
                                  in_=encT_r[c, :, ts(tg, 512)])

        kT_sb = proj.tile([128, TK], DT_QK, tag="kT")
        v_sb = proj.tile([128, KC, D], DT_E, tag="v")

        pv = [psPV.tile([128, 1024], F32, tag="ps_pv", name=f"pv{g}")
              for g in range(NG)]
        racc = [rpool.tile([128, 1024], F32, tag="racc", name=f"racc{g}")
                for g in range(NG)]

        def attention_unit(g, k_glob):
            s12 = psS.tile([128, 1024], F32, tag="ps_s", name="s12")
            nc.tensor.matmul(s12[:, 0:512],
                             lhsT=kT_sb[0:64, ts(k_glob, 128)],
                             rhs=qT_sb[0:64, ts(g, 512)],
                             start=True, stop=True, tile_position=(0, 0))
            nc.tensor.matmul(s12[:, 512:1024],
                             lhsT=kT_sb[64:128, ts(k_glob, 128)],
                             rhs=qT_sb[64:128, ts(g, 512)],
                             start=True, stop=True, tile_position=(64, 0))
            e12 = epool.tile([128, 1024], DT_E, tag="e", name="e12")
            nc.scalar.activation(e12, s12, Exp, scale=SCALE)
            for h in range(2):
                nc.tensor.matmul(pv[g][:, ts(h, 512)],
                                 lhsT=v_sb[:, k_glob, :],
                                 rhs=e12[:, ts(h, 512)],
                                 start=(k_glob == 0), stop=(k_glob == KC - 1),
                                 skip_group_check=True)
            if k_glob == 0:
                nc.vector.tensor_copy(racc[g], e12)
            else:
                nc.vector.tensor_add(racc[g], racc[g], e12)

        # ---- phase A: k/v projections interleaved with group-0 attention ----
        for tg in range(TKG):
            # k^T for this Tk group
            kp = psS.tile([128, 1024], F32, tag="ps_s")
            for c in range(EC):
                nc.tensor.matmul(kp[:, 0:512], lhsT=wk_sb[:, c, :],
                                 rhs=enc_sb[:, c, ts(tg, 512)],
                                 start=(c == 0), stop=(c == EC - 1))
            nc.scalar.add(kT_sb[:, ts(tg, 512)], kp[:, 0:512], add=bk_sb)

            # v (natural) for this group: 4 blocks of [128, 128]
            for t in range(4):
                tk = tg * 4 + t
                vp = psS.tile([128, 1024], F32, tag="ps_s")
                if with_vbias:
                    nc.tensor.matmul(vp[:, 0:D], lhsT=ones_row_f32, rhs=bv_sb,
                                     start=True, stop=False)
                for c in range(EC):
                    nc.tensor.matmul(vp[:, 0:D],
                                     lhsT=enc_sb[:, c, ts(tk, 128)],
                                     rhs=wv_sb[:, c, :],
                                     start=(not with_vbias and c == 0),
                                     stop=(c == EC - 1))
                nc.scalar.copy(v_sb[:, tk, :], vp[:, 0:D])

            for kc in range(4):
                attention_unit(0, tg * 4 + kc)

        # ---- phase B: group-1 attention (k/v staged) ----
        for k_glob in range(KC):
            attention_unit(1, k_glob)

        # ---- row sums + stream A and r out; normalize happens on host ----
        outp = ctx.enter_context(tc.tile_pool(name="outp", bufs=2))
        for g in range(NG):
            r12p = psS.tile([1, 1024], F32, tag="ps_s")
            for h in range(2):
                nc.tensor.matmul(r12p[:, ts(h, 512)], lhsT=ones_col,
                                 rhs=racc[g][:, ts(h, 512)], start=True, stop=True)
            r_sb = outp.tile([1, 1024], F32, tag="r_sb")
            nc.vector.tensor_copy(r_sb, r12p)
            nc.sync.dma_start(out=rd[g, :].rearrange("(o t) -> o t", o=1),
                              in_=r_sb)
            pv_sb = outp.tile([128, 1024], F32, tag="pv_sb")
            nc.scalar.copy(pv_sb, pv[g])
            nc.sync.dma_start(out=pvd[:, ts(g, 1024)], in_=pv_sb)

    return nc


_nc_cache = {}


def _make_bass(with_vbias: bool):
    from concourse import bacc

    nc = bacc.Bacc("TRN2", target_bir_lowering=False, debug=False)
    _build(nc, with_vbias)
    nc.compile()
    return nc


def kernel(x, encoder_out, W_q, b_q, W_k, b_k, W_v, b_v,
           lambda_q1, lambda_k1, lambda_q2, lambda_k2, lambda_init):
    from concourse import bass_utils

    x = np.asarray(x, np.float32)
    encoder_out = np.asarray(encoder_out, np.float32)
    W_q = np.asarray(W_q, np.float32).astype(DT_IN)
    W_k = np.asarray(W_k, np.float32).astype(DT_IN)
    W_v = np.asarray(W_v, np.float32).astype(DT_IN)
    b_q = np.asarray(b_q, np.float32)
    b_k = np.asarray(b_k, np.float32)
    b_v = np.asarray(b_v, np.float32)

    lam = np.float32(
        np.exp(np.float32(np.asarray(lambda_q1, np.float32)
                          @ np.asarray(lambda_k1, np.float32)))
        - np.exp(np.float32(np.asarray(lambda_q2, np.float32)
                            @ np.asarray(lambda_k2, np.float32)))
        + np.float32(np.asarray(lambda_init, np.float32))
    )

    with_vbias = bool(np.any(b_v))
    if with_vbias not in _nc_cache:
        _nc_cache[with_vbias] = _make_bass(with_vbias)
    nc = _nc_cache[with_vbias]

    encTs = [np.ascontiguousarray(encoder_out[b].T).astype(DT_IN) for b in range(B)]
    in_maps = []
    for c in range(NCORES):
        b, h = divmod(c, 2)
        xTs = np.ascontiguousarray(x[b, h * TQL:(h + 1) * TQL, :].T).astype(DT_IN)
        in_maps.append({
            "xT": xTs, "encT": encTs[b],
            "wq": W_q, "wk": W_k, "wv": W_v,
            "bq": b_q, "bk": b_k, "bv": b_v,
        })

    res = bass_utils.run_bass_kernel_spmd(nc, in_maps, core_ids=list(range(NCORES)))
    kernel.last_result = res

    out = np.empty((B, TQ, D), np.float32)
    for c in range(NCORES):
        b, h = divmod(c, 2)
        pvd = res.results[c]["pvd"]          # [D, NG*1024]
        rd = res.results[c]["rd"]            # [NG, 1024]
        for g in range(NG):
            A = pvd[:, g * 1024:(g + 1) * 1024]
            A1, A2 = A[:, 0:512], A[:, 512:1024]
            r1, r2 = rd[g, 0:512], rd[g, 512:1024]
            o = A1 / r1 - lam * (A2 / r2)    # [D, 512]
            q0 = h * TQL + g * 512
            out[b, q0:q0 + 512, :] = o.T
    return out
